# revision 1
# baseline (speedup 1.0000x reference)
# Trainium2 Bass kernel for nn_CustomStyleLoss (segment-mean + MSE reduction).
#
# loss = sum_rows mean_chunks( (mean_chunk(input) - mean_chunk(style))^2 )
# with rows = 16*512 = 8192, each row = 50*50 = 2500 elems = 25 chunks of 100.
#
# Data-parallel over the row axis: core i gets rows [i*1024, (i+1)*1024).
# Per core: 8 tiles of [128 rows x 2500 f32] per tensor, streamed HBM->SBUF.
# DVE computes diff and per-chunk sums; ACT squares with the scale folded in
# (scale = 1/(100*sqrt(25)) = 0.002, so (0.002*chunk_sum)^2 sums directly to
# the loss) and accumulates over the 25 chunks into one column of a [128 x 8]
# partials tile. One 4KB DMA returns partials; host sums all partials.

import sys

if "/opt/trn_rl_repo" not in sys.path:
    sys.path.insert(0, "/opt/trn_rl_repo")

import numpy as np

import concourse.bacc as bacc
import concourse.tile as tile
from concourse import mybir
from concourse.bass_utils import run_bass_kernel_spmd

N_CORES = 8
N_ROWS = 8192          # 16 * 512
K = 2500               # 50 * 50
CHUNK = 100
N_CHUNKS = K // CHUNK  # 25
P = 128
ROWS_PER_CORE = N_ROWS // N_CORES   # 1024
N_TILES = ROWS_PER_CORE // P        # 8
# (0.002 * chunk_sum)^2 == chunk_sum^2 / 100^2 / 25  ->  summing these over
# chunks/rows/cores gives the loss directly.
SCALE = 1.0 / (CHUNK * np.sqrt(N_CHUNKS))

_CACHED_NC = None


def _build_nc():
    nc = bacc.Bacc(
        "TRN2",
        target_bir_lowering=False,
        debug=False,
        num_devices=N_CORES,
    )
    x = nc.dram_tensor(
        "input", [ROWS_PER_CORE, K], mybir.dt.float32, kind="ExternalInput"
    ).ap()
    s = nc.dram_tensor(
        "style", [ROWS_PER_CORE, K], mybir.dt.float32, kind="ExternalInput"
    ).ap()
    o = nc.dram_tensor(
        "out", [P, N_TILES], mybir.dt.float32, kind="ExternalOutput"
    ).ap()

    with tile.TileContext(nc) as tc:
        with (
            tc.tile_pool(name="io", bufs=4) as io_pool,
            tc.tile_pool(name="work", bufs=3) as work_pool,
            tc.tile_pool(name="stats", bufs=1) as stats_pool,
        ):
            partials = stats_pool.tile([P, N_TILES], mybir.dt.float32)
            for t in range(N_TILES):
                xt = io_pool.tile([P, K], mybir.dt.float32, tag="xt")
                st = io_pool.tile([P, K], mybir.dt.float32, tag="st")
                nc.sync.dma_start(out=xt, in_=x[t * P : (t + 1) * P, :])
                nc.sync.dma_start(out=st, in_=s[t * P : (t + 1) * P, :])

                d = work_pool.tile([P, K], mybir.dt.float32, tag="d")
                nc.vector.tensor_sub(d, xt, st)

                cs = work_pool.tile([P, N_CHUNKS], mybir.dt.float32, tag="cs")
                nc.vector.tensor_reduce(
                    out=cs,
                    in_=d.rearrange("p (c k) -> p c k", k=CHUNK),
                    axis=mybir.AxisListType.X,
                    op=mybir.AluOpType.add,
                )

                sq = work_pool.tile([P, N_CHUNKS], mybir.dt.float32, tag="sq")
                nc.scalar.activation(
                    out=sq,
                    in_=cs,
                    func=mybir.ActivationFunctionType.Square,
                    scale=float(SCALE),
                    accum_out=partials[:, t : t + 1],
                )
            nc.sync.dma_start(out=o, in_=partials)
    nc.compile()
    return nc


def _get_nc():
    global _CACHED_NC
    if _CACHED_NC is None:
        _CACHED_NC = _build_nc()
    return _CACHED_NC


def run_sharded(input, style, **run_kwargs):
    """Shard, run on 8 cores, return (scalar loss, BassKernelResults)."""
    nc = _get_nc()
    xi = np.ascontiguousarray(np.asarray(input, dtype=np.float32)).reshape(
        N_ROWS, K
    )
    xs = np.ascontiguousarray(np.asarray(style, dtype=np.float32)).reshape(
        N_ROWS, K
    )
    in_maps = [
        {
            "input": xi[i * ROWS_PER_CORE : (i + 1) * ROWS_PER_CORE],
            "style": xs[i * ROWS_PER_CORE : (i + 1) * ROWS_PER_CORE],
        }
        for i in range(N_CORES)
    ]
    res = run_bass_kernel_spmd(nc, in_maps, list(range(N_CORES)), **run_kwargs)
    total = np.float64(0.0)
    for r in res.results:
        total += r["out"].astype(np.float64).sum()
    return np.array(total, dtype=np.float32), res


def kernel(input, style):
    loss, _ = run_sharded(input, style)
    return loss
